# revision 30
# baseline (speedup 1.0000x reference)
"""Paged sparse-attention (prefill + paged prefix) Trainium2 kernel.

Sharding: tensor-parallel over KV heads — 8 KV heads across 8 NeuronCores.
Each core handles 1 KV head and its 4 GQA query heads for all 4 sequences.
No collectives needed (heads are independent); host concatenates outputs.

Math: reference = LSE-merge of (causal attn over new tokens) and (non-causal
attn over paged prefix) == single softmax over concatenated [new; prefix]
keys with a causal mask on the new-token block. Scores are small (|s| <~ 6)
so max-subtraction is skipped (exp cannot overflow); the causal mask is a
0/1 multiply on the two diagonal 128-blocks after exp.

Host prep: K/V are gathered per sequence (new tokens first, then the paged
prefix via block_table), transposed as needed, and cast to bf16 on the host;
K is additionally pre-scaled by A = scale*128/ln2 so the PWL exp on DVE is a
single ADD in the bf16-bit domain and ScalarE's exp just uses scale=ln2/128.

Per core, per sequence b, per 128-key chunk j (S^T layout: keys on
partitions, (g, s) query columns folded to nq=1024):
  S^T[j]  = K_chunk_j @ Q'^T        (bf16 matmuls, K^T chunk stationary)
  P^T[j]  = exp-ish(S^T[j])         split: cols 0:XS on ScalarE (LUT exp),
                                    cols XS: on DVE (bits = s~ + B, int16
                                    reinterpreted as bf16; ~3% rel err)
  O[m]   += P^T[j][:, m-chunk].T @ [V_j | 1]  (ones col => softmax denom)
  out[m]  = O[m][:, :128] / O[m][:, 128]

The PE stream is software-pipelined one chunk deep: S(0), S(1), PV(0),
S(2), PV(1), ... so score matmuls for chunk j+1 run while chunk j's exp is
in flight on ScalarE/DVE; PV(j) then starts right as exp(j) completes. The
two 4-m-slot PSUM accumulators (2 banks each) let the next sequence's first
PV half start as soon as the matching normalize half has read its banks.
"""

import numpy as np
import ml_dtypes

from concourse import bacc
import concourse.mybir as mybir
import concourse.tile as tile
from concourse.tile_rust import add_dep_helper
from concourse.bass_utils import run_bass_kernel_spmd

# Problem shape (hardcoded per harness contract)
HQ, HKV, DH, PAGE = 32, 8, 128, 16
B, S, PREFIX = 4, 256, 2048
N = B * S                      # 1024 new tokens
NSLOTS = 16384
G = HQ // HKV                  # 4 query heads per kv head
NQ = G * S                     # 1024 query columns per sequence per core
L = PREFIX + S                 # 2304 keys per sequence
JCH = L // 128                 # 18 key chunks of 128 (0,1 = new; 2.. = prefix)
MCH = NQ // 128                # 8 query chunks of 128
SCALE = DH ** -0.5
NCORES = 8

FEXP_A = float(SCALE * 128.0 / np.log(2.0))          # host pre-scale on K
FEXP_B = float(127.0 * 128.0 - 366393.0 / 65536.0)   # PWL exp bias (bits)
SEXP = float(np.log(2.0) / 128.0)                    # ScalarE exp scale
# Chunk processing order: the two causal-masked new-token chunks (0, 1) go
# at positions 2-3 so their extra DVE mask multiplies land after the
# sequence-boundary normalize has drained from the DVE queue.
ORDER = [2, 3, 0, 1] + list(range(4, JCH))
# exp is split per chunk: score cols [0:XS] on ScalarE (LUT exp, ~780ns),
# cols [XS:] on DVE (PWL bits trick, ~3% rel err, ~460ns) — both fit inside
# the PE's per-chunk score+PV window so PV(j) never waits. Position 0 (the
# first chunk after a sequence switch) runs entirely on ScalarE so the DVE
# queue is free to drain the previous sequence's normalize.
XS = 576
# positions whose exp runs entirely on ScalarE (DVE drains normalize there)
FULL_ACT_POS = frozenset({0})

F32 = mybir.dt.float32
BF16 = mybir.dt.bfloat16
I16 = mybir.dt.int16

# DMA split points (chunk aligned), in ISSUE order: descriptor writes cost
# ~650ns each on the issuing queue, so pieces are few; the piece holding the
# first-processed chunks (2, 3) goes first. V rides the GpSimd ring so K/Q
# own the SP ring.
K_PIECES = [(256, 512), (512, 1024), (0, 256), (1024, 2304)]
V_PIECES = [(0, 4), (4, 11), (11, 18)]
PREP_POS = 8                   # position at which next seq's loads go


def build_bass():
    nc = bacc.Bacc(trn_type="TRN2")

    qT = nc.dram_tensor("qT", [DH, B * NQ], BF16, kind="ExternalInput")
    kTd = nc.dram_tensor("kTd", [DH, B * L], BF16, kind="ExternalInput")
    vdd = nc.dram_tensor("vd", [B * JCH * 128, DH], BF16, kind="ExternalInput")
    maskd = nc.dram_tensor("maskd", [128, 128], BF16, kind="ExternalInput")
    out = nc.dram_tensor("out", [B * MCH * 128, DH], BF16, kind="ExternalOutput")

    with tile.TileContext(nc) as tc:
        with (
            tc.tile_pool(name="singles", bufs=1) as singles,
            tc.tile_pool(name="kv", bufs=2) as kv,
            tc.tile_pool(name="pp", bufs=4) as pp,
            tc.tile_pool(name="outp", bufs=4) as outp,
            tc.tile_pool(name="small", bufs=8) as small,
            tc.tile_pool(name="ps_s", bufs=2, space="PSUM") as ps_s,
            tc.tile_pool(name="ps_o", bufs=2, space="PSUM") as ps_o,
        ):
            # mask rides the ACT HW-DGE ring so Q/K/V own the SP ring
            mask_sb = singles.tile([128, 128], BF16)
            nc.scalar.dma_start(mask_sb[:], maskd[:, :])

            # force the EXP ACT-table load NOW, before seq-0's K pieces
            # queue up on the ACT ring — otherwise the first real exp waits
            # ~1.3us for the table mid-prologue
            dum = singles.tile([128, 1], F32)
            nc.vector.memset(dum[:], 0.0)
            nc.scalar.activation(
                out=dum[:], in_=dum[:],
                func=mybir.ActivationFunctionType.Exp, scale=1.0,
            )

            def prep(b):
                """Issue all loads for sequence b. Inputs are bf16 and
                host-packed per sequence, so every DMA is a plain slab;
                PE reads the DMA'd tiles directly (waits are absorbed into
                tensor nops at first use)."""
                qt = kv.tile([DH, NQ], BF16, tag="qt")
                kt = kv.tile([DH, L], BF16, tag="kt")
                va = kv.tile([128, JCH, DH + 1], BF16, tag="va")
                dmas = {}

                def kdma(ring, a, z):
                    dmas["k", a] = ring.dma_start(
                        kt[:, a:z], kTd[:, b * L + a : b * L + z]
                    )

                def vdma(ring, c0, c1):
                    r0, r1 = (b * JCH + c0) * 128, (b * JCH + c1) * 128
                    dmas["v", c0] = ring.dma_start(
                        va[:, c0:c1, :DH],
                        vdd[r0:r1, :].rearrange("(c p) d -> p c d", p=128),
                    )

                dmas["ms"] = nc.vector.memset(va[:, :, DH : DH + 1], 1.0)
                if b == 0:
                    # boot: K on the ACT ring, Q/V on the SP ring so the two
                    # rings issue and transfer in parallel
                    dmas["q0"] = nc.sync.dma_start(
                        qt[:, :512], qT[:, b * NQ : b * NQ + 512]
                    )
                    for a, z in K_PIECES:
                        kdma(nc.scalar, a, z)
                    dmas["q1"] = nc.sync.dma_start(
                        qt[:, 512:], qT[:, b * NQ + 512 : (b + 1) * NQ]
                    )
                    for c0, c1 in V_PIECES:
                        vdma(nc.sync, c0, c1)
                else:
                    # steady state: everything on the SP ring (out-stores
                    # ride gpsimd), in order of first use by the PE
                    dmas["q0"] = nc.sync.dma_start(
                        qt[:, :512], qT[:, b * NQ : b * NQ + 512]
                    )
                    kdma(nc.sync, *K_PIECES[0])
                    dmas["q1"] = nc.sync.dma_start(
                        qt[:, 512:], qT[:, b * NQ + 512 : (b + 1) * NQ]
                    )
                    kdma(nc.sync, *K_PIECES[1])
                    kdma(nc.sync, *K_PIECES[2])
                    vdma(nc.sync, *V_PIECES[0])
                    kdma(nc.sync, *K_PIECES[3])
                    vdma(nc.sync, *V_PIECES[1])
                    vdma(nc.sync, *V_PIECES[2])
                return qt, kt, va, dmas

            preps = {0: prep(0)}

            # PE_HAM clock-gate warmup: open the gate to 2.4 GHz during the
            # queue-boot + first-DMA window so real chunks start at speed.
            warm = singles.tile([128, 512], BF16)
            nc.vector.memset(warm[:], 0.0)
            for _ in range(5):
                pw = ps_s.tile([128, NQ], F32, tag="ps")
                nc.tensor.matmul(
                    pw[:, :512], lhsT=warm[:, :128], rhs=warm[:],
                    start=True, stop=True,
                )

            kpiece_of = {}
            for a, z in K_PIECES:
                for c in range(a // 128, z // 128):
                    kpiece_of[c] = a
            vpiece_of = {}
            for c0, c1 in V_PIECES:
                for c in range(c0, c1):
                    vpiece_of[c] = c0

            norm_instrs = []   # prev seq's normalize reads of the po banks
            ps_hist = []       # per score-chunk: exp instrs reading its ps

            for b in range(B):
                qt, kt, va, dmas = preps.pop(b)
                po = [
                    ps_o.tile([128, 4, 256], F32, tag="po", name="poA"),
                    ps_o.tile([128, 4, 256], F32, tag="po", name="poB"),
                ]
                seen_k, seen_v = set(), set()
                pend = []

                def emit_pv(pos, j, pt, deps, po=po, va=va):
                    # Waits (exp completion, V DMA, po WAR) ride a nop so
                    # the PV LDWEIGHTS stay wait-free and HW weight
                    # prefetch overlaps the previous matmul.
                    pvnop = nc.tensor.nop(nofuse=True)
                    for d in deps:
                        add_dep_helper(
                            pvnop.ins, d.ins, sync=True, reason="pv gate"
                        )
                    prev_mm = None
                    for m in range(MCH):
                        if j == 1 and m % 2 == 0:
                            # keys 128..255 are fully masked for s < 128
                            continue
                        mm = nc.tensor.matmul(
                            po[m // 4][:, m % 4, : DH + 1],
                            lhsT=pt[:, m * 128 : (m + 1) * 128],
                            rhs=va[:, j, :],
                            start=(pos == 0 and m % 2 == 0),
                            stop=(pos == JCH - 1),
                            skip_group_check=True,
                        )
                        if pos == 0:
                            # start=True clears has_written for the WHOLE
                            # bank; odd m relies on the even m's clear and
                            # is order-pinned behind it.
                            if m % 2 == 1 and prev_mm is not None:
                                add_dep_helper(
                                    mm.ins, prev_mm.ins, sync=False,
                                    reason="has_written bank clear order",
                                )
                            prev_mm = mm

                for pos, j in enumerate(ORDER):
                    if pos == PREP_POS and b + 1 < B:
                        preps[b + 1] = prep(b + 1)

                    ps = ps_s.tile([128, NQ], F32, tag="ps")
                    # Input-DMA waits are absorbed into a tensor nop so the
                    # score LDWEIGHTS stay wait-free (a wait on the LDW
                    # blocks HW weight prefetch even when long satisfied).
                    # The ps-slot WAR (exp of pos-2) needs no explicit dep:
                    # the pv nop of pos-2 already waited on that exp
                    # earlier in the PE queue.
                    wdeps = []
                    if len(ps_hist) >= 2:
                        wdeps += ps_hist[-2]
                    q1dep = None
                    if pos == 0:
                        wdeps.append(dmas["q0"])
                        q1dep = dmas["q1"]
                    kp = kpiece_of[j]
                    if kp not in seen_k:
                        seen_k.add(kp)
                        wdeps.append(dmas["k", kp])
                    if wdeps:
                        wnop = nc.tensor.nop(nofuse=True)
                        for d in wdeps:
                            add_dep_helper(
                                wnop.ins, d.ins, sync=True,
                                reason="score input gate",
                            )

                    pt = pp.tile([128, NQ], BF16, tag="pt")
                    if j == 1:
                        # even-m fully masked: scores/exp for odd-m cols only
                        qodd = qt.rearrange(
                            "p (g h q) -> p g h q", g=4, h=2
                        )[:, :, 1, :]
                        nc.tensor.matmul(
                            ps[:, :512], lhsT=kt[:, 128:256], rhs=qodd,
                            start=True, stop=True,
                        )
                        pt4 = pt.rearrange("p (g h q) -> p g h q", g=4, h=2)
                        e1 = nc.scalar.activation(
                            out=pt4[:, :3, 1, :], in_=ps[:, :384],
                            func=mybir.ActivationFunctionType.Exp,
                            scale=SEXP,
                        )
                        e2 = nc.vector.tensor_scalar_add(
                            pt[:, 896:1024].bitcast(I16), ps[:, 384:512],
                            FEXP_B,
                        )
                        tt = nc.vector.tensor_tensor(
                            pt4[:, :, 1, :], pt4[:, :, 1, :],
                            mask_sb[:, None, :].to_broadcast((128, 4, 128)),
                            mybir.AluOpType.mult,
                        )
                        deps_pv = [e1, tt]
                    else:
                        for h2 in range(2):
                            if h2 == 1 and q1dep is not None:
                                qnop = nc.tensor.nop(nofuse=True)
                                add_dep_helper(
                                    qnop.ins, q1dep.ins, sync=True,
                                    reason="q half B gate",
                                )
                            nc.tensor.matmul(
                                ps[:, h2 * 512 : (h2 + 1) * 512],
                                lhsT=kt[:, j * 128 : (j + 1) * 128],
                                rhs=qt[:, h2 * 512 : (h2 + 1) * 512],
                                start=True, stop=True,
                            )
                        if pos in FULL_ACT_POS:
                            # whole chunk on ScalarE: the DVE queue is busy
                            # with the previous sequence's normalize
                            e1 = nc.scalar.activation(
                                out=pt[:, :], in_=ps[:],
                                func=mybir.ActivationFunctionType.Exp,
                                scale=SEXP,
                            )
                            deps_pv = [e1]
                        else:
                            e1 = nc.scalar.activation(
                                out=pt[:, :XS], in_=ps[:, :XS],
                                func=mybir.ActivationFunctionType.Exp,
                                scale=SEXP,
                            )
                            e2 = nc.vector.tensor_scalar_add(
                                pt[:, XS:].bitcast(I16), ps[:, XS:], FEXP_B
                            )
                            deps_pv = [e1, e2]
                        if j == 0:
                            pt4 = pt.rearrange(
                                "p (g h q) -> p g h q", g=4, h=2
                            )
                            tt = nc.vector.tensor_tensor(
                                pt4[:, :, 0, :], pt4[:, :, 0, :],
                                mask_sb[:, None, :].to_broadcast(
                                    (128, 4, 128)
                                ),
                                mybir.AluOpType.mult,
                            )
                            deps_pv = deps_pv + [tt]

                    ps_hist.append([e1] if (
                        pos in FULL_ACT_POS and j != 1
                    ) else [e1, e2])

                    if len(pend) >= 2:
                        emit_pv(*pend.pop(0))

                    extra = []
                    vp = vpiece_of[j]
                    if vp not in seen_v:
                        seen_v.add(vp)
                        extra.append(dmas["v", vp])
                    if pos == 0:
                        extra.append(dmas["ms"])
                        extra += norm_instrs
                        norm_instrs = []
                    pend.append((pos, j, pt, deps_pv + extra))
                for p_ in pend:
                    emit_pv(*p_)
                pend = []

                # normalize: o = po[:, :, :128] / po[:, :, 128], in halves so
                # the first store overlaps the second divide
                nrm = []
                for hv in range(2):
                    dinv = small.tile([128, 4, 1], F32, tag="dinv")
                    rc = nc.vector.reciprocal(
                        dinv[:], po[hv][:, :, DH : DH + 1]
                    )
                    osb = outp.tile([128, 4, DH], BF16, tag="osb")
                    tt = nc.vector.tensor_tensor(
                        osb[:], po[hv][:, :, :DH],
                        dinv.to_broadcast([128, 4, DH]),
                        mybir.AluOpType.mult,
                    )
                    r0 = b * NQ + hv * 512
                    oring = nc.gpsimd if b < B - 1 else nc.sync
                    oring.dma_start(
                        out[r0 : r0 + 512, :].rearrange(
                            "(m p) d -> p m d", p=128
                        ),
                        osb[:],
                    )
                    nrm += [rc, tt]
                norm_instrs = nrm
    nc.finalize()
    return nc


def _prepare(q, k, v, k_cache, v_cache, slot_mapping, block_table):
    """Host-side shard prep: KV-cache scatter, per-sequence gather (new
    tokens first, then the paged prefix), transpose, bf16 cast, and the
    PWL-exp pre-scale on K."""
    q = np.asarray(q, np.float32)
    k = np.asarray(k, np.float32)
    v = np.asarray(v, np.float32)
    k_cache = np.array(k_cache, np.float32)
    v_cache = np.array(v_cache, np.float32)
    slot_mapping = np.asarray(slot_mapping, np.int64)
    block_table = np.asarray(block_table, np.int64)

    k_cache[slot_mapping] = k
    v_cache[slot_mapping] = v

    slot_idx = (
        block_table[:, :, None] * PAGE + np.arange(PAGE, dtype=np.int64)
    ).reshape(B, PREFIX)

    # keys per sequence in kernel order: [new (256) | prefix (2048)]
    kg = np.concatenate([k.reshape(B, S, -1), k_cache[slot_idx]], axis=1)
    vg = np.concatenate([v.reshape(B, S, -1), v_cache[slot_idx]], axis=1)

    # the causal mask reduces to ONE lower-triangular [128,128] block
    mask = np.triu(np.ones((128, 128))).astype(ml_dtypes.bfloat16)
    BF = ml_dtypes.bfloat16

    in_maps = []
    for h in range(NCORES):
        sl = slice(h * DH, (h + 1) * DH)
        qh = q[:, h * G * DH : (h + 1) * G * DH]
        qTh = (
            qh.reshape(B, S, G, DH).transpose(3, 0, 2, 1)
            .reshape(DH, B * NQ).astype(BF)
        )
        kTh = (
            (FEXP_A * kg[:, :, sl]).transpose(2, 0, 1)
            .reshape(DH, B * L).astype(BF)
        )
        vdh = vg[:, :, sl].reshape(B * L, DH).astype(BF)
        in_maps.append(
            dict(
                qT=np.ascontiguousarray(qTh),
                kTd=np.ascontiguousarray(kTh),
                vd=np.ascontiguousarray(vdh),
                maskd=mask,
            )
        )
    return in_maps


def _assemble(results):
    """results: per-core dicts with 'out' [B*MCH*128, DH] rows=(b, m, qp),
    m = g*2 + s_half. Returns [N, HQ*DH]."""
    full = np.empty((N, HQ * DH), np.float32)
    for h, res in enumerate(results):
        o = np.asarray(res["out"], np.float32).reshape(B, G, 2, 128, DH)
        oc = o.transpose(0, 2, 3, 1, 4).reshape(N, G * DH)  # (b, s)(g, d)
        full[:, h * G * DH : (h + 1) * G * DH] = oc
    return full


def _ensure_ntff_hook():
    """The image's `antenv` stub lacks `axon_hooks`; register the same
    ctypes-based NTFF profile hook trn_agent_boot would have installed so
    trace=True / BASS_TRACE=1 profiling works."""
    try:
        import antenv.axon_hooks  # noqa: F401
        return
    except ImportError:
        pass
    import sys
    import types

    mod = types.ModuleType("antenv.axon_hooks")
    mod._hook = None
    mod.set_axon_ntff_profile_hook = lambda h: setattr(mod, "_hook", h)
    mod.get_axon_ntff_profile_hook = lambda: mod._hook
    sys.modules["antenv.axon_hooks"] = mod
    import antenv

    antenv.axon_hooks = mod
    try:
        from trn_agent_boot.trn_boot import _ntff_profile_via_ctypes

        mod._hook = _ntff_profile_via_ctypes("/opt/axon/libaxon_pjrt.so")
    except Exception:
        mod._hook = None


def run(trace=False, **inputs):
    _ensure_ntff_hook()
    in_maps = _prepare(**inputs)
    nc = build_bass()
    res = run_bass_kernel_spmd(
        nc, in_maps, core_ids=list(range(NCORES)), trace=trace
    )
    return _assemble(res.results), res


def kernel(**inputs) -> np.ndarray:
    out, _ = run(trace=False, **inputs)
    return out


# revision 31
# speedup vs baseline: 1.0277x; 1.0277x over previous
"""Paged sparse-attention (prefill + paged prefix) Trainium2 kernel.

Sharding: tensor-parallel over KV heads — 8 KV heads across 8 NeuronCores.
Each core handles 1 KV head and its 4 GQA query heads for all 4 sequences.
No collectives needed (heads are independent); host concatenates outputs.

Math: reference = LSE-merge of (causal attn over new tokens) and (non-causal
attn over paged prefix) == single softmax over concatenated [new; prefix]
keys with a causal mask on the new-token block. Scores are small (|s| <~ 6)
so max-subtraction is skipped (exp cannot overflow); the causal mask is a
0/1 multiply on the two diagonal 128-blocks after exp.

Host prep: K/V are gathered per sequence (new tokens first, then the paged
prefix via block_table), transposed as needed, and cast to bf16 on the host;
K is additionally pre-scaled by A = scale*128/ln2 so the PWL exp on DVE is a
single ADD in the bf16-bit domain and ScalarE's exp just uses scale=ln2/128.

Per core, per sequence b, per 128-key chunk j (S^T layout: keys on
partitions, (g, s) query columns folded to nq=1024):
  S^T[j]  = K_chunk_j @ Q'^T        (bf16 matmuls, K^T chunk stationary)
  P^T[j]  = exp-ish(S^T[j])         split: cols 0:XS on ScalarE (LUT exp),
                                    cols XS: on DVE (bits = s~ + B, int16
                                    reinterpreted as bf16; ~3% rel err)
  O[m]   += P^T[j][:, m-chunk].T @ [V_j | 1]  (ones col => softmax denom)
  out[m]  = O[m][:, :128] / O[m][:, 128]

The PE stream is software-pipelined one chunk deep: S(0), S(1), PV(0),
S(2), PV(1), ... so score matmuls for chunk j+1 run while chunk j's exp is
in flight on ScalarE/DVE; PV(j) then starts right as exp(j) completes. The
two 4-m-slot PSUM accumulators (2 banks each) let the next sequence's first
PV half start as soon as the matching normalize half has read its banks.
"""

import numpy as np
import ml_dtypes

from concourse import bacc
import concourse.mybir as mybir
import concourse.tile as tile
from concourse.tile_rust import add_dep_helper
from concourse.bass_utils import run_bass_kernel_spmd

# Problem shape (hardcoded per harness contract)
HQ, HKV, DH, PAGE = 32, 8, 128, 16
B, S, PREFIX = 4, 256, 2048
N = B * S                      # 1024 new tokens
NSLOTS = 16384
G = HQ // HKV                  # 4 query heads per kv head
NQ = G * S                     # 1024 query columns per sequence per core
L = PREFIX + S                 # 2304 keys per sequence
JCH = L // 128                 # 18 key chunks of 128 (0,1 = new; 2.. = prefix)
MCH = NQ // 128                # 8 query chunks of 128
SCALE = DH ** -0.5
NCORES = 8

FEXP_A = float(SCALE * 128.0 / np.log(2.0))          # host pre-scale on K
FEXP_B = float(127.0 * 128.0 - 366393.0 / 65536.0)   # PWL exp bias (bits)
SEXP = float(np.log(2.0) / 128.0)                    # ScalarE exp scale
# Chunk processing order: the two causal-masked new-token chunks (0, 1) go
# at positions 2-3 so their extra DVE mask multiplies land after the
# sequence-boundary normalize has drained from the DVE queue.
ORDER = [2, 3, 0, 1] + list(range(4, JCH))
# exp is split per chunk: score cols [0:XS] on ScalarE (LUT exp, ~780ns),
# cols [XS:] on DVE (PWL bits trick, ~3% rel err, ~460ns) — both fit inside
# the PE's per-chunk score+PV window so PV(j) never waits. Position 0 (the
# first chunk after a sequence switch) runs entirely on ScalarE so the DVE
# queue is free to drain the previous sequence's normalize.
XS = 576
# positions whose exp runs entirely on ScalarE (DVE drains normalize there)
FULL_ACT_POS = frozenset({0, 2})

F32 = mybir.dt.float32
BF16 = mybir.dt.bfloat16
I16 = mybir.dt.int16

# DMA split points (chunk aligned), in ISSUE order: descriptor writes cost
# ~650ns each on the issuing queue, so pieces are few; the piece holding the
# first-processed chunks (2, 3) goes first. V rides the GpSimd ring so K/Q
# own the SP ring.
K_PIECES = [(256, 1024), (0, 256), (1024, 2304)]
V_PIECES = [(0, 4), (4, 11), (11, 18)]
PREP_POS = 8                   # position at which next seq's loads go


def build_bass():
    nc = bacc.Bacc(trn_type="TRN2")

    qT = nc.dram_tensor("qT", [DH, B * NQ], BF16, kind="ExternalInput")
    kTd = nc.dram_tensor("kTd", [DH, B * L], BF16, kind="ExternalInput")
    vdd = nc.dram_tensor("vd", [B * JCH * 128, DH], BF16, kind="ExternalInput")
    maskd = nc.dram_tensor("maskd", [128, 128], BF16, kind="ExternalInput")
    out = nc.dram_tensor("out", [B * MCH * 128, DH], BF16, kind="ExternalOutput")

    with tile.TileContext(nc) as tc:
        with (
            tc.tile_pool(name="singles", bufs=1) as singles,
            tc.tile_pool(name="kv", bufs=2) as kv,
            tc.tile_pool(name="pp", bufs=4) as pp,
            tc.tile_pool(name="outp", bufs=4) as outp,
            tc.tile_pool(name="small", bufs=8) as small,
            tc.tile_pool(name="ps_s", bufs=2, space="PSUM") as ps_s,
            tc.tile_pool(name="ps_o", bufs=2, space="PSUM") as ps_o,
        ):
            # mask rides the ACT HW-DGE ring so Q/K/V own the SP ring
            mask_sb = singles.tile([128, 128], BF16)
            nc.scalar.dma_start(mask_sb[:], maskd[:, :])

            # force the EXP ACT-table load NOW, before seq-0's K pieces
            # queue up on the ACT ring — otherwise the first real exp waits
            # ~1.3us for the table mid-prologue
            dum = singles.tile([128, 1], F32)
            nc.vector.memset(dum[:], 0.0)
            nc.scalar.activation(
                out=dum[:], in_=dum[:],
                func=mybir.ActivationFunctionType.Exp, scale=1.0,
            )

            def prep(b):
                """Issue all loads for sequence b. Inputs are bf16 and
                host-packed per sequence, so every DMA is a plain slab;
                PE reads the DMA'd tiles directly (waits are absorbed into
                tensor nops at first use)."""
                qt = kv.tile([DH, NQ], BF16, tag="qt")
                kt = kv.tile([DH, L], BF16, tag="kt")
                va = kv.tile([128, JCH, DH + 1], BF16, tag="va")
                dmas = {}

                def kdma(ring, a, z):
                    dmas["k", a] = ring.dma_start(
                        kt[:, a:z], kTd[:, b * L + a : b * L + z]
                    )

                def vdma(ring, c0, c1):
                    r0, r1 = (b * JCH + c0) * 128, (b * JCH + c1) * 128
                    dmas["v", c0] = ring.dma_start(
                        va[:, c0:c1, :DH],
                        vdd[r0:r1, :].rearrange("(c p) d -> p c d", p=128),
                    )

                dmas["ms"] = nc.vector.memset(va[:, :, DH : DH + 1], 1.0)
                if b == 0:
                    # boot: K on the ACT ring, Q/V on the SP ring so the two
                    # rings issue and transfer in parallel
                    dmas["q0"] = nc.sync.dma_start(
                        qt[:, :512], qT[:, b * NQ : b * NQ + 512]
                    )
                    for a, z in K_PIECES:
                        kdma(nc.scalar, a, z)
                    dmas["q1"] = nc.sync.dma_start(
                        qt[:, 512:], qT[:, b * NQ + 512 : (b + 1) * NQ]
                    )
                    for c0, c1 in V_PIECES:
                        vdma(nc.sync, c0, c1)
                else:
                    # steady state: everything on the SP ring (out-stores
                    # ride gpsimd), in order of first use by the PE
                    dmas["q0"] = nc.sync.dma_start(
                        qt[:, :512], qT[:, b * NQ : b * NQ + 512]
                    )
                    kdma(nc.sync, *K_PIECES[0])
                    dmas["q1"] = nc.sync.dma_start(
                        qt[:, 512:], qT[:, b * NQ + 512 : (b + 1) * NQ]
                    )
                    kdma(nc.sync, *K_PIECES[1])
                    vdma(nc.sync, *V_PIECES[0])
                    kdma(nc.sync, *K_PIECES[2])
                    vdma(nc.sync, *V_PIECES[1])
                    vdma(nc.sync, *V_PIECES[2])
                return qt, kt, va, dmas

            preps = {0: prep(0)}

            # PE_HAM clock-gate warmup: open the gate to 2.4 GHz during the
            # queue-boot + first-DMA window so real chunks start at speed.
            warm = singles.tile([128, 512], BF16)
            nc.vector.memset(warm[:], 0.0)
            for _ in range(5):
                pw = ps_s.tile([128, NQ], F32, tag="ps")
                nc.tensor.matmul(
                    pw[:, :512], lhsT=warm[:, :128], rhs=warm[:],
                    start=True, stop=True,
                )

            kpiece_of = {}
            for a, z in K_PIECES:
                for c in range(a // 128, z // 128):
                    kpiece_of[c] = a
            vpiece_of = {}
            for c0, c1 in V_PIECES:
                for c in range(c0, c1):
                    vpiece_of[c] = c0

            norm_instrs = []   # prev seq's normalize reads of the po banks
            ps_hist = []       # per score-chunk: exp instrs reading its ps

            for b in range(B):
                qt, kt, va, dmas = preps.pop(b)
                po = [
                    ps_o.tile([128, 4, 256], F32, tag="po", name="poA"),
                    ps_o.tile([128, 4, 256], F32, tag="po", name="poB"),
                ]
                seen_k, seen_v = set(), set()
                pend = []

                def emit_pv(pos, j, pt, deps, po=po, va=va):
                    # Waits (exp completion, V DMA, po WAR) ride a nop so
                    # the PV LDWEIGHTS stay wait-free and HW weight
                    # prefetch overlaps the previous matmul.
                    pvnop = nc.tensor.nop(nofuse=True)
                    for d in deps:
                        add_dep_helper(
                            pvnop.ins, d.ins, sync=True, reason="pv gate"
                        )
                    prev_mm = None
                    for m in range(MCH):
                        if j == 1 and m % 2 == 0:
                            # keys 128..255 are fully masked for s < 128
                            continue
                        mm = nc.tensor.matmul(
                            po[m // 4][:, m % 4, : DH + 1],
                            lhsT=pt[:, m * 128 : (m + 1) * 128],
                            rhs=va[:, j, :],
                            start=(pos == 0 and m % 2 == 0),
                            stop=(pos == JCH - 1),
                            skip_group_check=True,
                        )
                        if pos == 0:
                            # start=True clears has_written for the WHOLE
                            # bank; odd m relies on the even m's clear and
                            # is order-pinned behind it.
                            if m % 2 == 1 and prev_mm is not None:
                                add_dep_helper(
                                    mm.ins, prev_mm.ins, sync=False,
                                    reason="has_written bank clear order",
                                )
                            prev_mm = mm

                for pos, j in enumerate(ORDER):
                    if pos == PREP_POS and b + 1 < B:
                        preps[b + 1] = prep(b + 1)

                    ps = ps_s.tile([128, NQ], F32, tag="ps")
                    # Input-DMA waits are absorbed into a tensor nop so the
                    # score LDWEIGHTS stay wait-free (a wait on the LDW
                    # blocks HW weight prefetch even when long satisfied).
                    # The ps-slot WAR (exp of pos-2) needs no explicit dep:
                    # the pv nop of pos-2 already waited on that exp
                    # earlier in the PE queue.
                    wdeps = []
                    if len(ps_hist) >= 2:
                        wdeps += ps_hist[-2]
                    q1dep = None
                    if pos == 0:
                        wdeps.append(dmas["q0"])
                        q1dep = dmas["q1"]
                    kp = kpiece_of[j]
                    if kp not in seen_k:
                        seen_k.add(kp)
                        wdeps.append(dmas["k", kp])
                    if wdeps:
                        wnop = nc.tensor.nop(nofuse=True)
                        for d in wdeps:
                            add_dep_helper(
                                wnop.ins, d.ins, sync=True,
                                reason="score input gate",
                            )

                    pt = pp.tile([128, NQ], BF16, tag="pt")
                    if j == 1:
                        # even-m fully masked: scores/exp for odd-m cols only
                        qodd = qt.rearrange(
                            "p (g h q) -> p g h q", g=4, h=2
                        )[:, :, 1, :]
                        nc.tensor.matmul(
                            ps[:, :512], lhsT=kt[:, 128:256], rhs=qodd,
                            start=True, stop=True,
                        )
                        pt4 = pt.rearrange("p (g h q) -> p g h q", g=4, h=2)
                        e1 = nc.scalar.activation(
                            out=pt4[:, :3, 1, :], in_=ps[:, :384],
                            func=mybir.ActivationFunctionType.Exp,
                            scale=SEXP,
                        )
                        e2 = nc.vector.tensor_scalar_add(
                            pt[:, 896:1024].bitcast(I16), ps[:, 384:512],
                            FEXP_B,
                        )
                        tt = nc.vector.tensor_tensor(
                            pt4[:, :, 1, :], pt4[:, :, 1, :],
                            mask_sb[:, None, :].to_broadcast((128, 4, 128)),
                            mybir.AluOpType.mult,
                        )
                        deps_pv = [e1, tt]
                    else:
                        for h2 in range(2):
                            if h2 == 1 and q1dep is not None:
                                qnop = nc.tensor.nop(nofuse=True)
                                add_dep_helper(
                                    qnop.ins, q1dep.ins, sync=True,
                                    reason="q half B gate",
                                )
                            nc.tensor.matmul(
                                ps[:, h2 * 512 : (h2 + 1) * 512],
                                lhsT=kt[:, j * 128 : (j + 1) * 128],
                                rhs=qt[:, h2 * 512 : (h2 + 1) * 512],
                                start=True, stop=True,
                            )
                        if pos in FULL_ACT_POS:
                            # whole chunk on ScalarE: the DVE queue is busy
                            # with the previous sequence's normalize
                            e1 = nc.scalar.activation(
                                out=pt[:, :], in_=ps[:],
                                func=mybir.ActivationFunctionType.Exp,
                                scale=SEXP,
                            )
                            deps_pv = [e1]
                        else:
                            e1 = nc.scalar.activation(
                                out=pt[:, :XS], in_=ps[:, :XS],
                                func=mybir.ActivationFunctionType.Exp,
                                scale=SEXP,
                            )
                            e2 = nc.vector.tensor_scalar_add(
                                pt[:, XS:].bitcast(I16), ps[:, XS:], FEXP_B
                            )
                            deps_pv = [e1, e2]
                        if j == 0:
                            pt4 = pt.rearrange(
                                "p (g h q) -> p g h q", g=4, h=2
                            )
                            tt = nc.vector.tensor_tensor(
                                pt4[:, :, 0, :], pt4[:, :, 0, :],
                                mask_sb[:, None, :].to_broadcast(
                                    (128, 4, 128)
                                ),
                                mybir.AluOpType.mult,
                            )
                            deps_pv = deps_pv + [tt]

                    ps_hist.append([e1] if (
                        pos in FULL_ACT_POS and j != 1
                    ) else [e1, e2])

                    if len(pend) >= 2:
                        emit_pv(*pend.pop(0))

                    extra = []
                    vp = vpiece_of[j]
                    if vp not in seen_v:
                        seen_v.add(vp)
                        extra.append(dmas["v", vp])
                    if pos == 0:
                        extra.append(dmas["ms"])
                        extra += norm_instrs
                        norm_instrs = []
                    pend.append((pos, j, pt, deps_pv + extra))
                for p_ in pend:
                    emit_pv(*p_)
                pend = []

                # normalize: o = po[:, :, :128] / po[:, :, 128], in halves so
                # the first store overlaps the second divide
                nrm = []
                for hv in range(2):
                    dinv = small.tile([128, 4, 1], F32, tag="dinv")
                    rc = nc.vector.reciprocal(
                        dinv[:], po[hv][:, :, DH : DH + 1]
                    )
                    osb = outp.tile([128, 4, DH], BF16, tag="osb")
                    tt = nc.vector.tensor_tensor(
                        osb[:], po[hv][:, :, :DH],
                        dinv.to_broadcast([128, 4, DH]),
                        mybir.AluOpType.mult,
                    )
                    r0 = b * NQ + hv * 512
                    oring = nc.gpsimd if b < B - 1 else nc.sync
                    oring.dma_start(
                        out[r0 : r0 + 512, :].rearrange(
                            "(m p) d -> p m d", p=128
                        ),
                        osb[:],
                    )
                    nrm += [rc, tt]
                norm_instrs = nrm
    nc.finalize()
    return nc


def _prepare(q, k, v, k_cache, v_cache, slot_mapping, block_table):
    """Host-side shard prep: KV-cache scatter, per-sequence gather (new
    tokens first, then the paged prefix), transpose, bf16 cast, and the
    PWL-exp pre-scale on K."""
    q = np.asarray(q, np.float32)
    k = np.asarray(k, np.float32)
    v = np.asarray(v, np.float32)
    k_cache = np.array(k_cache, np.float32)
    v_cache = np.array(v_cache, np.float32)
    slot_mapping = np.asarray(slot_mapping, np.int64)
    block_table = np.asarray(block_table, np.int64)

    k_cache[slot_mapping] = k
    v_cache[slot_mapping] = v

    slot_idx = (
        block_table[:, :, None] * PAGE + np.arange(PAGE, dtype=np.int64)
    ).reshape(B, PREFIX)

    # keys per sequence in kernel order: [new (256) | prefix (2048)]
    kg = np.concatenate([k.reshape(B, S, -1), k_cache[slot_idx]], axis=1)
    vg = np.concatenate([v.reshape(B, S, -1), v_cache[slot_idx]], axis=1)

    # the causal mask reduces to ONE lower-triangular [128,128] block
    mask = np.triu(np.ones((128, 128))).astype(ml_dtypes.bfloat16)
    BF = ml_dtypes.bfloat16

    in_maps = []
    for h in range(NCORES):
        sl = slice(h * DH, (h + 1) * DH)
        qh = q[:, h * G * DH : (h + 1) * G * DH]
        qTh = (
            qh.reshape(B, S, G, DH).transpose(3, 0, 2, 1)
            .reshape(DH, B * NQ).astype(BF)
        )
        kTh = (
            (FEXP_A * kg[:, :, sl]).transpose(2, 0, 1)
            .reshape(DH, B * L).astype(BF)
        )
        vdh = vg[:, :, sl].reshape(B * L, DH).astype(BF)
        in_maps.append(
            dict(
                qT=np.ascontiguousarray(qTh),
                kTd=np.ascontiguousarray(kTh),
                vd=np.ascontiguousarray(vdh),
                maskd=mask,
            )
        )
    return in_maps


def _assemble(results):
    """results: per-core dicts with 'out' [B*MCH*128, DH] rows=(b, m, qp),
    m = g*2 + s_half. Returns [N, HQ*DH]."""
    full = np.empty((N, HQ * DH), np.float32)
    for h, res in enumerate(results):
        o = np.asarray(res["out"], np.float32).reshape(B, G, 2, 128, DH)
        oc = o.transpose(0, 2, 3, 1, 4).reshape(N, G * DH)  # (b, s)(g, d)
        full[:, h * G * DH : (h + 1) * G * DH] = oc
    return full


def _ensure_ntff_hook():
    """The image's `antenv` stub lacks `axon_hooks`; register the same
    ctypes-based NTFF profile hook trn_agent_boot would have installed so
    trace=True / BASS_TRACE=1 profiling works."""
    try:
        import antenv.axon_hooks  # noqa: F401
        return
    except ImportError:
        pass
    import sys
    import types

    mod = types.ModuleType("antenv.axon_hooks")
    mod._hook = None
    mod.set_axon_ntff_profile_hook = lambda h: setattr(mod, "_hook", h)
    mod.get_axon_ntff_profile_hook = lambda: mod._hook
    sys.modules["antenv.axon_hooks"] = mod
    import antenv

    antenv.axon_hooks = mod
    try:
        from trn_agent_boot.trn_boot import _ntff_profile_via_ctypes

        mod._hook = _ntff_profile_via_ctypes("/opt/axon/libaxon_pjrt.so")
    except Exception:
        mod._hook = None


def run(trace=False, **inputs):
    _ensure_ntff_hook()
    in_maps = _prepare(**inputs)
    nc = build_bass()
    res = run_bass_kernel_spmd(
        nc, in_maps, core_ids=list(range(NCORES)), trace=trace
    )
    return _assemble(res.results), res


def kernel(**inputs) -> np.ndarray:
    out, _ = run(trace=False, **inputs)
    return out


# revision 32
# speedup vs baseline: 1.0441x; 1.0160x over previous
"""Paged sparse-attention (prefill + paged prefix) Trainium2 kernel.

Sharding: tensor-parallel over KV heads — 8 KV heads across 8 NeuronCores.
Each core handles 1 KV head and its 4 GQA query heads for all 4 sequences.
No collectives needed (heads are independent); host concatenates outputs.

Math: reference = LSE-merge of (causal attn over new tokens) and (non-causal
attn over paged prefix) == single softmax over concatenated [new; prefix]
keys with a causal mask on the new-token block. Scores are small (|s| <~ 6)
so max-subtraction is skipped (exp cannot overflow); the causal mask is a
0/1 multiply on the two diagonal 128-blocks after exp.

Host prep: K/V are gathered per sequence (new tokens first, then the paged
prefix via block_table), transposed as needed, and cast to bf16 on the host;
K is additionally pre-scaled by A = scale*128/ln2 so the PWL exp on DVE is a
single ADD in the bf16-bit domain and ScalarE's exp just uses scale=ln2/128.

Per core, per sequence b, per 128-key chunk j (S^T layout: keys on
partitions, (g, s) query columns folded to nq=1024):
  S^T[j]  = K_chunk_j @ Q'^T        (bf16 matmuls, K^T chunk stationary)
  P^T[j]  = exp-ish(S^T[j])         split: cols 0:XS on ScalarE (LUT exp),
                                    cols XS: on DVE (bits = s~ + B, int16
                                    reinterpreted as bf16; ~3% rel err)
  O[m]   += P^T[j][:, m-chunk].T @ [V_j | 1]  (ones col => softmax denom)
  out[m]  = O[m][:, :128] / O[m][:, 128]

The PE stream is software-pipelined one chunk deep: S(0), S(1), PV(0),
S(2), PV(1), ... so score matmuls for chunk j+1 run while chunk j's exp is
in flight on ScalarE/DVE; PV(j) then starts right as exp(j) completes. The
two 4-m-slot PSUM accumulators (2 banks each) let the next sequence's first
PV half start as soon as the matching normalize half has read its banks.
"""

import numpy as np
import ml_dtypes

from concourse import bacc
import concourse.mybir as mybir
import concourse.tile as tile
from concourse.tile_rust import add_dep_helper
from concourse.bass_utils import run_bass_kernel_spmd

# Problem shape (hardcoded per harness contract)
HQ, HKV, DH, PAGE = 32, 8, 128, 16
B, S, PREFIX = 4, 256, 2048
N = B * S                      # 1024 new tokens
NSLOTS = 16384
G = HQ // HKV                  # 4 query heads per kv head
NQ = G * S                     # 1024 query columns per sequence per core
L = PREFIX + S                 # 2304 keys per sequence
JCH = L // 128                 # 18 key chunks of 128 (0,1 = new; 2.. = prefix)
MCH = NQ // 128                # 8 query chunks of 128
SCALE = DH ** -0.5
NCORES = 8

FEXP_A = float(SCALE * 128.0 / np.log(2.0))          # host pre-scale on K
FEXP_B = float(127.0 * 128.0 - 366393.0 / 65536.0)   # PWL exp bias (bits)
SEXP = float(np.log(2.0) / 128.0)                    # ScalarE exp scale
# Chunk processing order: the two causal-masked new-token chunks (0, 1) go
# at positions 2-3 so their extra DVE mask multiplies land after the
# sequence-boundary normalize has drained from the DVE queue.
ORDER = [2, 3, 0, 1] + list(range(4, JCH))
# exp is split per chunk: score cols [0:XS] on ScalarE (LUT exp, ~780ns),
# cols [XS:] on DVE (PWL bits trick, ~3% rel err, ~460ns) — both fit inside
# the PE's per-chunk score+PV window so PV(j) never waits. Position 0 (the
# first chunk after a sequence switch) runs entirely on ScalarE so the DVE
# queue is free to drain the previous sequence's normalize.
XS = 576
# positions whose exp runs entirely on ScalarE (DVE drains normalize there)
FULL_ACT_POS = frozenset({0, 2})

F32 = mybir.dt.float32
BF16 = mybir.dt.bfloat16
I16 = mybir.dt.int16

# DMA split points (chunk aligned), in ISSUE order: descriptor writes cost
# ~650ns each on the issuing queue, so pieces are few; the piece holding the
# first-processed chunks (2, 3) goes first. V rides the GpSimd ring so K/Q
# own the SP ring.
K_PIECES = [(256, 1024), (0, 256), (1024, 2304)]
V_PIECES = [(0, 4), (4, 11), (11, 18)]
PREP_POS = 8                   # position at which next seq's loads go


def build_bass():
    nc = bacc.Bacc(trn_type="TRN2")

    qT = nc.dram_tensor("qT", [DH, B * NQ], BF16, kind="ExternalInput")
    kTd = nc.dram_tensor("kTd", [DH, B * L], BF16, kind="ExternalInput")
    vdd = nc.dram_tensor("vd", [B * JCH * 128, DH], BF16, kind="ExternalInput")
    maskd = nc.dram_tensor("maskd", [128, 128], BF16, kind="ExternalInput")
    out = nc.dram_tensor("out", [B * MCH * 128, DH], BF16, kind="ExternalOutput")

    with tile.TileContext(nc) as tc:
        with (
            tc.tile_pool(name="singles", bufs=1) as singles,
            tc.tile_pool(name="kv", bufs=2) as kv,
            tc.tile_pool(name="pp", bufs=4) as pp,
            tc.tile_pool(name="outp", bufs=4) as outp,
            tc.tile_pool(name="small", bufs=8) as small,
            tc.tile_pool(name="ps_s", bufs=2, space="PSUM") as ps_s,
            tc.tile_pool(name="ps_o", bufs=2, space="PSUM") as ps_o,
        ):
            # mask rides the ACT HW-DGE ring so Q/K/V own the SP ring
            mask_sb = singles.tile([128, 128], BF16)
            nc.scalar.dma_start(mask_sb[:], maskd[:, :])

            # force the EXP ACT-table load NOW, before seq-0's K pieces
            # queue up on the ACT ring — otherwise the first real exp waits
            # ~1.3us for the table mid-prologue
            dum = singles.tile([128, 1], F32)
            nc.vector.memset(dum[:], 0.0)
            nc.scalar.activation(
                out=dum[:], in_=dum[:],
                func=mybir.ActivationFunctionType.Exp, scale=1.0,
            )

            def prep(b):
                """Issue all loads for sequence b. Inputs are bf16 and
                host-packed per sequence, so every DMA is a plain slab;
                PE reads the DMA'd tiles directly (waits are absorbed into
                tensor nops at first use)."""
                qt = kv.tile([DH, NQ], BF16, tag="qt")
                kt = kv.tile([DH, L], BF16, tag="kt")
                va = kv.tile([128, JCH, DH + 1], BF16, tag="va")
                dmas = {}

                def kdma(ring, a, z):
                    dmas["k", a] = ring.dma_start(
                        kt[:, a:z], kTd[:, b * L + a : b * L + z]
                    )

                def vdma(ring, c0, c1):
                    r0, r1 = (b * JCH + c0) * 128, (b * JCH + c1) * 128
                    dmas["v", c0] = ring.dma_start(
                        va[:, c0:c1, :DH],
                        vdd[r0:r1, :].rearrange("(c p) d -> p c d", p=128),
                    )

                dmas["ms"] = nc.vector.memset(va[:, :, DH : DH + 1], 1.0)
                if b == 0:
                    # boot: the ACT ring moves only 64KB (cols 256:512 — all
                    # that positions 0-1 need) before compute can start; the
                    # rest of K's first piece rides the SP ring behind Q and
                    # gates position 4
                    dmas["q0"] = nc.sync.dma_start(
                        qt[:, :512], qT[:, b * NQ : b * NQ + 512]
                    )
                    dmas["k", 256] = nc.scalar.dma_start(
                        kt[:, 256:512], kTd[:, b * L + 256 : b * L + 512]
                    )
                    dmas["q1"] = nc.sync.dma_start(
                        qt[:, 512:], qT[:, b * NQ + 512 : (b + 1) * NQ]
                    )
                    dmas["k_mid"] = nc.sync.dma_start(
                        kt[:, 512:1024], kTd[:, b * L + 512 : b * L + 1024]
                    )
                    kdma(nc.scalar, *K_PIECES[1])
                    kdma(nc.scalar, *K_PIECES[2])
                    for c0, c1 in V_PIECES:
                        vdma(nc.sync, c0, c1)
                else:
                    # steady state: everything on the SP ring (out-stores
                    # ride gpsimd), in order of first use by the PE
                    dmas["q0"] = nc.sync.dma_start(
                        qt[:, :512], qT[:, b * NQ : b * NQ + 512]
                    )
                    kdma(nc.sync, *K_PIECES[0])
                    dmas["q1"] = nc.sync.dma_start(
                        qt[:, 512:], qT[:, b * NQ + 512 : (b + 1) * NQ]
                    )
                    kdma(nc.sync, *K_PIECES[1])
                    vdma(nc.sync, *V_PIECES[0])
                    kdma(nc.sync, *K_PIECES[2])
                    vdma(nc.sync, *V_PIECES[1])
                    vdma(nc.sync, *V_PIECES[2])
                return qt, kt, va, dmas

            preps = {0: prep(0)}

            # PE_HAM clock-gate warmup: open the gate to 2.4 GHz during the
            # queue-boot + first-DMA window so real chunks start at speed.
            warm = singles.tile([128, 512], BF16)
            nc.vector.memset(warm[:], 0.0)
            for _ in range(5):
                pw = ps_s.tile([128, NQ], F32, tag="ps")
                nc.tensor.matmul(
                    pw[:, :512], lhsT=warm[:, :128], rhs=warm[:],
                    start=True, stop=True,
                )

            kpiece_of = {}
            for a, z in K_PIECES:
                for c in range(a // 128, z // 128):
                    kpiece_of[c] = a
            vpiece_of = {}
            for c0, c1 in V_PIECES:
                for c in range(c0, c1):
                    vpiece_of[c] = c0

            norm_instrs = []   # prev seq's normalize reads of the po banks
            ps_hist = []       # per score-chunk: exp instrs reading its ps

            for b in range(B):
                qt, kt, va, dmas = preps.pop(b)
                po = [
                    ps_o.tile([128, 4, 256], F32, tag="po", name="poA"),
                    ps_o.tile([128, 4, 256], F32, tag="po", name="poB"),
                ]
                seen_k, seen_v = set(), set()
                pend = []

                def emit_pv(pos, j, pt, deps, po=po, va=va):
                    # Waits (exp completion, V DMA, po WAR) ride a nop so
                    # the PV LDWEIGHTS stay wait-free and HW weight
                    # prefetch overlaps the previous matmul.
                    pvnop = nc.tensor.nop(nofuse=True)
                    for d in deps:
                        add_dep_helper(
                            pvnop.ins, d.ins, sync=True, reason="pv gate"
                        )
                    prev_mm = None
                    for m in range(MCH):
                        if j == 1 and m % 2 == 0:
                            # keys 128..255 are fully masked for s < 128
                            continue
                        mm = nc.tensor.matmul(
                            po[m // 4][:, m % 4, : DH + 1],
                            lhsT=pt[:, m * 128 : (m + 1) * 128],
                            rhs=va[:, j, :],
                            start=(pos == 0 and m % 2 == 0),
                            stop=(pos == JCH - 1),
                            skip_group_check=True,
                        )
                        if pos == 0:
                            # start=True clears has_written for the WHOLE
                            # bank; odd m relies on the even m's clear and
                            # is order-pinned behind it.
                            if m % 2 == 1 and prev_mm is not None:
                                add_dep_helper(
                                    mm.ins, prev_mm.ins, sync=False,
                                    reason="has_written bank clear order",
                                )
                            prev_mm = mm

                for pos, j in enumerate(ORDER):
                    if pos == PREP_POS and b + 1 < B:
                        preps[b + 1] = prep(b + 1)

                    ps = ps_s.tile([128, NQ], F32, tag="ps")
                    # Input-DMA waits are absorbed into a tensor nop so the
                    # score LDWEIGHTS stay wait-free (a wait on the LDW
                    # blocks HW weight prefetch even when long satisfied).
                    # The ps-slot WAR (exp of pos-2) needs no explicit dep:
                    # the pv nop of pos-2 already waited on that exp
                    # earlier in the PE queue.
                    wdeps = []
                    if len(ps_hist) >= 2:
                        wdeps += ps_hist[-2]
                    q1dep = None
                    if pos == 0:
                        wdeps.append(dmas["q0"])
                        q1dep = dmas["q1"]
                    kp = kpiece_of[j]
                    if kp not in seen_k:
                        seen_k.add(kp)
                        wdeps.append(dmas["k", kp])
                    if pos == 4 and "k_mid" in dmas:
                        wdeps.append(dmas.pop("k_mid"))
                    if wdeps:
                        wnop = nc.tensor.nop(nofuse=True)
                        for d in wdeps:
                            add_dep_helper(
                                wnop.ins, d.ins, sync=True,
                                reason="score input gate",
                            )

                    pt = pp.tile([128, NQ], BF16, tag="pt")
                    if j == 1:
                        # even-m fully masked: scores/exp for odd-m cols only
                        qodd = qt.rearrange(
                            "p (g h q) -> p g h q", g=4, h=2
                        )[:, :, 1, :]
                        nc.tensor.matmul(
                            ps[:, :512], lhsT=kt[:, 128:256], rhs=qodd,
                            start=True, stop=True,
                        )
                        pt4 = pt.rearrange("p (g h q) -> p g h q", g=4, h=2)
                        e1 = nc.scalar.activation(
                            out=pt4[:, :3, 1, :], in_=ps[:, :384],
                            func=mybir.ActivationFunctionType.Exp,
                            scale=SEXP,
                        )
                        e2 = nc.vector.tensor_scalar_add(
                            pt[:, 896:1024].bitcast(I16), ps[:, 384:512],
                            FEXP_B,
                        )
                        tt = nc.vector.tensor_tensor(
                            pt4[:, :, 1, :], pt4[:, :, 1, :],
                            mask_sb[:, None, :].to_broadcast((128, 4, 128)),
                            mybir.AluOpType.mult,
                        )
                        deps_pv = [e1, tt]
                    else:
                        for h2 in range(2):
                            if h2 == 1 and q1dep is not None:
                                qnop = nc.tensor.nop(nofuse=True)
                                add_dep_helper(
                                    qnop.ins, q1dep.ins, sync=True,
                                    reason="q half B gate",
                                )
                            nc.tensor.matmul(
                                ps[:, h2 * 512 : (h2 + 1) * 512],
                                lhsT=kt[:, j * 128 : (j + 1) * 128],
                                rhs=qt[:, h2 * 512 : (h2 + 1) * 512],
                                start=True, stop=True,
                            )
                        if pos in FULL_ACT_POS:
                            # whole chunk on ScalarE: the DVE queue is busy
                            # with the previous sequence's normalize
                            e1 = nc.scalar.activation(
                                out=pt[:, :], in_=ps[:],
                                func=mybir.ActivationFunctionType.Exp,
                                scale=SEXP,
                            )
                            deps_pv = [e1]
                        else:
                            e1 = nc.scalar.activation(
                                out=pt[:, :XS], in_=ps[:, :XS],
                                func=mybir.ActivationFunctionType.Exp,
                                scale=SEXP,
                            )
                            e2 = nc.vector.tensor_scalar_add(
                                pt[:, XS:].bitcast(I16), ps[:, XS:], FEXP_B
                            )
                            deps_pv = [e1, e2]
                        if j == 0:
                            pt4 = pt.rearrange(
                                "p (g h q) -> p g h q", g=4, h=2
                            )
                            tt = nc.vector.tensor_tensor(
                                pt4[:, :, 0, :], pt4[:, :, 0, :],
                                mask_sb[:, None, :].to_broadcast(
                                    (128, 4, 128)
                                ),
                                mybir.AluOpType.mult,
                            )
                            deps_pv = deps_pv + [tt]

                    ps_hist.append([e1] if (
                        pos in FULL_ACT_POS and j != 1
                    ) else [e1, e2])

                    if len(pend) >= 2:
                        emit_pv(*pend.pop(0))

                    extra = []
                    vp = vpiece_of[j]
                    if vp not in seen_v:
                        seen_v.add(vp)
                        extra.append(dmas["v", vp])
                    if pos == 0:
                        extra.append(dmas["ms"])
                        extra += norm_instrs
                        norm_instrs = []
                    pend.append((pos, j, pt, deps_pv + extra))
                for p_ in pend:
                    emit_pv(*p_)
                pend = []

                # normalize: o = po[:, :, :128] / po[:, :, 128], in halves so
                # the first store overlaps the second divide
                nrm = []
                for hv in range(2):
                    dinv = small.tile([128, 4, 1], F32, tag="dinv")
                    rc = nc.vector.reciprocal(
                        dinv[:], po[hv][:, :, DH : DH + 1]
                    )
                    osb = outp.tile([128, 4, DH], BF16, tag="osb")
                    tt = nc.vector.tensor_tensor(
                        osb[:], po[hv][:, :, :DH],
                        dinv.to_broadcast([128, 4, DH]),
                        mybir.AluOpType.mult,
                    )
                    r0 = b * NQ + hv * 512
                    oring = nc.gpsimd if b < B - 1 else nc.sync
                    oring.dma_start(
                        out[r0 : r0 + 512, :].rearrange(
                            "(m p) d -> p m d", p=128
                        ),
                        osb[:],
                    )
                    nrm += [rc, tt]
                norm_instrs = nrm
    nc.finalize()
    return nc


def _prepare(q, k, v, k_cache, v_cache, slot_mapping, block_table):
    """Host-side shard prep: KV-cache scatter, per-sequence gather (new
    tokens first, then the paged prefix), transpose, bf16 cast, and the
    PWL-exp pre-scale on K."""
    q = np.asarray(q, np.float32)
    k = np.asarray(k, np.float32)
    v = np.asarray(v, np.float32)
    k_cache = np.array(k_cache, np.float32)
    v_cache = np.array(v_cache, np.float32)
    slot_mapping = np.asarray(slot_mapping, np.int64)
    block_table = np.asarray(block_table, np.int64)

    k_cache[slot_mapping] = k
    v_cache[slot_mapping] = v

    slot_idx = (
        block_table[:, :, None] * PAGE + np.arange(PAGE, dtype=np.int64)
    ).reshape(B, PREFIX)

    # keys per sequence in kernel order: [new (256) | prefix (2048)]
    kg = np.concatenate([k.reshape(B, S, -1), k_cache[slot_idx]], axis=1)
    vg = np.concatenate([v.reshape(B, S, -1), v_cache[slot_idx]], axis=1)

    # the causal mask reduces to ONE lower-triangular [128,128] block
    mask = np.triu(np.ones((128, 128))).astype(ml_dtypes.bfloat16)
    BF = ml_dtypes.bfloat16

    in_maps = []
    for h in range(NCORES):
        sl = slice(h * DH, (h + 1) * DH)
        qh = q[:, h * G * DH : (h + 1) * G * DH]
        qTh = (
            qh.reshape(B, S, G, DH).transpose(3, 0, 2, 1)
            .reshape(DH, B * NQ).astype(BF)
        )
        kTh = (
            (FEXP_A * kg[:, :, sl]).transpose(2, 0, 1)
            .reshape(DH, B * L).astype(BF)
        )
        vdh = vg[:, :, sl].reshape(B * L, DH).astype(BF)
        in_maps.append(
            dict(
                qT=np.ascontiguousarray(qTh),
                kTd=np.ascontiguousarray(kTh),
                vd=np.ascontiguousarray(vdh),
                maskd=mask,
            )
        )
    return in_maps


def _assemble(results):
    """results: per-core dicts with 'out' [B*MCH*128, DH] rows=(b, m, qp),
    m = g*2 + s_half. Returns [N, HQ*DH]."""
    full = np.empty((N, HQ * DH), np.float32)
    for h, res in enumerate(results):
        o = np.asarray(res["out"], np.float32).reshape(B, G, 2, 128, DH)
        oc = o.transpose(0, 2, 3, 1, 4).reshape(N, G * DH)  # (b, s)(g, d)
        full[:, h * G * DH : (h + 1) * G * DH] = oc
    return full


def _ensure_ntff_hook():
    """The image's `antenv` stub lacks `axon_hooks`; register the same
    ctypes-based NTFF profile hook trn_agent_boot would have installed so
    trace=True / BASS_TRACE=1 profiling works."""
    try:
        import antenv.axon_hooks  # noqa: F401
        return
    except ImportError:
        pass
    import sys
    import types

    mod = types.ModuleType("antenv.axon_hooks")
    mod._hook = None
    mod.set_axon_ntff_profile_hook = lambda h: setattr(mod, "_hook", h)
    mod.get_axon_ntff_profile_hook = lambda: mod._hook
    sys.modules["antenv.axon_hooks"] = mod
    import antenv

    antenv.axon_hooks = mod
    try:
        from trn_agent_boot.trn_boot import _ntff_profile_via_ctypes

        mod._hook = _ntff_profile_via_ctypes("/opt/axon/libaxon_pjrt.so")
    except Exception:
        mod._hook = None


def run(trace=False, **inputs):
    _ensure_ntff_hook()
    in_maps = _prepare(**inputs)
    nc = build_bass()
    res = run_bass_kernel_spmd(
        nc, in_maps, core_ids=list(range(NCORES)), trace=trace
    )
    return _assemble(res.results), res


def kernel(**inputs) -> np.ndarray:
    out, _ = run(trace=False, **inputs)
    return out
